# revision 38
# baseline (speedup 1.0000x reference)
"""Chamfer distance kernel for Trainium2 (Bass/Tile), 8 NeuronCores.

Problem: B=16 batches of point-cloud pairs (N=M=4096 points, 3-D).
  d[b,n,m] = |x1[b,n]|^2 + |x2[b,m]|^2 - 2*x1[b,n].x2[b,m]
  dist1/idx1 = min/argmin over m, dist2/idx2 = min/argmin over n.

Sharding: data-parallel over batch; each of the 8 cores handles 2 batches.

Device algorithm (per core, per batch, per orientation):
  Features A = -[x; y; z; 1; |p|^2], B = [-2x; -2y; -2z; |p|^2; 1] so a
  K-contracted matmul produces NEGATED squared distances s = -d. Each fp32
  feature is split exactly into 3 bf16 components and all 9 cross terms
  stacked along K (KF=45) so the bf16 matmul reproduces fp32-accurate
  products. PE writes s into [128, 2048] PSUM tiles (8 x 512-col matmuls
  per 128-row tile, 2 PSUM tiles ping-pong).

  The (otherwise idle) Scalar engine drains each PSUM tile into a full-row
  SBUF buffer [128, 4096] as FP16 (10-bit mantissa -> the upconverted fp32
  bits [0..12] are always zero, leaving room for a packed index field).
  The DVE does a position-preserving 4x fold with two stock tensor_tensor
  MAX ops — fp16 qualifies for the 2x_1p mode (2 elem/cycle) — then a
  single custom DVE op (ARGMIN_PACK_ANT) reduces the folded [128, 1024]
  row in ONE pass: key = (s & ~0x3FF) | (col & 0x3FF) packs value and
  column via BITWISE AND/XOR/OR (idx bits streamed as Src1 =
  bitcast(0x3F800000+col), mask as a raw-bit [P,1] scalar); a running
  float-max yields the min bucket and its first folded column k (IEEE
  ordering gives smallest-index tie-break for negative values). True
  column = k + 1024*j, j<4; the host resolves the 4 candidates and the
  output distances exactly from the original inputs (O(B*N)).

  Engine budget per core (measured): Scalar copies 524288 elem/lane at
  1 elem/cycle/1.2 GHz = 502 us and runs gap-free back-to-back (the
  pacer), PE streams bf16 matmul columns at a fixed 1.2 GHz (427 ns per
  512-col matmul, unrampable) = 437 us, DVE fold+scan = 431 us. The
  early-tile prefetch (NEARLY=4 row-tiles of lhs + full b0 rhs) keeps the
  PE fed while the big feature DMAs land. Pipeline lands at ~526 us vs
  1142 us baseline (599 us for the earlier no-fold fp32 scan variant);
  remaining span - scalar busy is ~24 us of fixed NEFF init/teardown.
"""

import sys

import numpy as np

for _p in ("/opt/trn_rl_repo", "/root/.axon_site/_ro/trn_rl_repo"):
    if _p not in sys.path:
        sys.path.append(_p)

B, N, M, D = 16, 4096, 4096, 3
NCORES = 8
BPC = B // NCORES          # batches per core
PT = 128                   # partition tile (output rows per matmul)
FC = 512                   # free-dim chunk (PSUM bank width in fp32)
HALF = 2048                # columns per PSUM tile
NT = N // PT               # 32 row tiles
KF = 45                    # 5 features x 9 bf16-split cross terms (exact fp32)
FOLD = 4                   # fold factor: scan M//FOLD cols, host resolves FOLD candidates
SCANW = M // FOLD          # 1024 columns scanned per row tile
IDXBITS = 10               # low mantissa bits carrying the column index
IDXMASK = (1 << IDXBITS) - 1

_CACHE = {}


def _register_op():
    import concourse.dve_ops as dops
    from concourse.dve_spec import Spec, Src0, Src1, C0, AluOp, Bin, maxx, MaxNeg

    for o in dops.OPS:
        if o.name == "ARGMIN_PACK_ANT":
            return o

    t = Bin(AluOp.BITWISE_AND, Src0, C0)
    u = Bin(AluOp.BITWISE_XOR, Src0, t)
    w = Bin(AluOp.BITWISE_AND, Src1, C0)
    key = Bin(AluOp.BITWISE_OR, u, w)

    def _ref(in0, in1, c0, c1, c2):
        v = np.ascontiguousarray(in0, np.float32).view(np.uint32)
        i = np.ascontiguousarray(in1, np.float32).view(np.uint32)
        m = np.ascontiguousarray(c0, np.float32).view(np.uint32).reshape(-1, 1)
        keyb = ((v & ~m) | (i & m)).view(np.float32)
        P = keyb.shape[0]
        acc = keyb.reshape(P, -1).max(axis=-1, keepdims=True)
        return keyb, acc

    spec = Spec(body=key, accum=maxx, accum_init=MaxNeg, reference=_ref)
    row = max(dops._SUB_OPCODE_FOR_NAME.values()) + 1
    assert row < 0x20
    op = dops.DveOp(
        "ARGMIN_PACK_ANT",
        spec,
        subdim=False,
        uops_sha={"v3": "bac1e8a7ec25ac9f", "v4": "4071d7dc865a2b50"},
    )
    dops.OPS.append(op)
    dops.CUSTOM_DVE_SPECS[op.name] = op.spec
    dops._SUB_OPCODE_FOR_NAME[op.name] = row
    return op


def _build_program():
    import concourse.mybir as mybir
    from concourse import bacc, tile
    from concourse.alu_op_type import AluOpType

    op = _register_op()

    fp32 = mybir.dt.float32
    bf16 = mybir.dt.bfloat16
    fp16 = mybir.dt.float16

    nc = bacc.Bacc(None, target_bir_lowering=False)

    a_dram = nc.dram_tensor("feat_a", [KF, BPC * N], bf16, kind="ExternalInput")
    b_dram = nc.dram_tensor("feat_b", [KF, BPC * M], bf16, kind="ExternalInput")
    idxf_dram = nc.dram_tensor("idxf", [PT, SCANW], fp32, kind="ExternalInput")
    mask_dram = nc.dram_tensor("maskc", [PT, 1], fp32, kind="ExternalInput")
    # packed argmin results: [bi*2+orient, partition, tile]
    pk_dram = nc.dram_tensor(
        "packed", [BPC * 2, PT, NT], fp32, kind="ExternalOutput"
    )

    with tile.TileContext(nc) as tc:
        with (
            tc.tile_pool(name="feat", bufs=1) as featp,
            tc.tile_pool(name="rows", bufs=4) as rowp,
            tc.tile_pool(name="fold", bufs=3) as foldp,
            tc.tile_pool(name="scr", bufs=3) as scrp,
            tc.tile_pool(name="res", bufs=3) as resp,
            tc.tile_pool(name="mm", bufs=2, space="PSUM") as mmp,
        ):
            a_sb = featp.tile([KF, BPC * N], bf16, tag="a")
            b_sb = featp.tile([KF, BPC * M], bf16, tag="b")
            idxf = featp.tile([PT, SCANW], fp32, tag="idxf")
            maskc = featp.tile([PT, 1], fp32, tag="maskc")
            # Small early tiles for the first FOUR row-tiles' matmuls, so PE
            # work starts ~1 us in and stays fed until the big feature DMAs
            # land (b0 is the full rhs for batch 0 / orientation 0).
            NEARLY = 4
            a0_sb = featp.tile([KF, NEARLY * PT], bf16, tag="a0")
            b0_sb = featp.tile([KF, M], bf16, tag="b0")
            # DMA order = need order: tile-0 operands first (PE can start
            # ~1us in), then the big feature tensors (needed from tile 1),
            # then the scan constants (first consumed ~20us in).
            nc.sync.dma_start(a0_sb[:], a_dram[:, 0:NEARLY * PT])
            # split b0 so the first matmul (needs cols 0:512 only) can
            # start as soon as the small leading chunk lands
            nc.sync.dma_start(b0_sb[:, 0:FC], b_dram[:, 0:FC])
            nc.sync.dma_start(b0_sb[:, FC:M], b_dram[:, FC:M])
            nc.sync.dma_start(a_sb[:], a_dram[:])
            nc.sync.dma_start(b_sb[:], b_dram[:])
            nc.sync.dma_start(idxf[:], idxf_dram[:])
            nc.sync.dma_start(maskc[:], mask_dram[:])

            # PE warmup: a few tiny matmuls so the tensor engine leaves its
            # cold p-state before the first real matmul group arrives.
            wl = featp.tile([1, PT], bf16, tag="warm_l")
            wr = featp.tile([1, FC], bf16, tag="warm_r")
            nc.gpsimd.memset(wl[:], 0)
            nc.gpsimd.memset(wr[:], 0)
            wm = mmp.tile([PT, HALF], fp32, tag="mm")
            for _ in range(3):
                nc.tensor.matmul(wm[:, 0:FC], wl[:], wr[:], start=True, stop=True)

            for bi in range(BPC):
                for orient in range(2):
                    if orient == 0:
                        lhs_all = a_sb[:, bi * N:(bi + 1) * N]
                        rhs_all = b_sb[:, bi * M:(bi + 1) * M]
                    else:
                        lhs_all = b_sb[:, bi * M:(bi + 1) * M]
                        rhs_all = a_sb[:, bi * N:(bi + 1) * N]

                    res = resp.tile([PT, NT], fp32, tag="res")

                    for t in range(NT):
                        if bi == 0 and orient == 0 and t < NEARLY:
                            lhsT = a0_sb[:, t * PT:(t + 1) * PT]
                            rhs_src = b0_sb
                        else:
                            lhsT = lhs_all[:, t * PT:(t + 1) * PT]
                            rhs_src = None
                        rowbuf = rowp.tile([PT, M], fp16, tag="rowbuf")
                        for h in range(2):
                            mm = mmp.tile([PT, HALF], fp32, tag="mm")
                            rsrc = rhs_src if rhs_src is not None else rhs_all
                            for q in range(4):
                                c = h * 4 + q
                                nc.tensor.matmul(
                                    mm[:, q * FC:(q + 1) * FC],
                                    lhsT,
                                    rsrc[:, c * FC:(c + 1) * FC],
                                    start=True,
                                    stop=True,
                                )
                            # idle Scalar engine drains PSUM -> SBUF fp16
                            # (10-bit mantissa: fp32 bits [0..12] stay zero,
                            # leaving room for the packed index field) and
                            # frees PSUM banks quickly for the PE.
                            nc.scalar.copy(
                                rowbuf[:, h * HALF:(h + 1) * HALF], mm[:]
                            )
                        # position-preserving 4x fold at the TT 2x_1p fp16
                        # rate (2 elem/cycle), then scan only M/4 columns.
                        # Final column = k + SCANW*j for j in 0..3; host
                        # resolves the 4 candidates exactly.
                        f1 = foldp.tile([PT, M // 2], fp16, tag="f1")
                        nc.vector.tensor_tensor(
                            f1[:],
                            rowbuf[:, 0:M // 2],
                            rowbuf[:, M // 2:M],
                            op=AluOpType.max,
                        )
                        f2 = foldp.tile([PT, SCANW], fp16, tag="f2")
                        nc.vector.tensor_tensor(
                            f2[:],
                            f1[:, 0:SCANW],
                            f1[:, SCANW:M // 2],
                            op=AluOpType.max,
                        )
                        scratch = scrp.tile([PT, SCANW], fp32, tag="scratch")
                        nc.vector._custom_dve(
                            op,
                            out=scratch[:],
                            in0=f2[:],
                            in1=idxf[:],
                            s0=maskc[:],
                            accum_out=res[:, t:t + 1],
                        )

                    nc.sync.dma_start(pk_dram[bi * 2 + orient], res[:])

    nc.compile()
    return nc


def _split3(x):
    """Exact 3-way bf16 decomposition of fp32: x == h + m + l."""
    import ml_dtypes

    bf = ml_dtypes.bfloat16
    h = x.astype(bf)
    r1 = (x - h.astype(np.float32)).astype(np.float32)
    m = r1.astype(bf)
    r2 = (r1 - m.astype(np.float32)).astype(np.float32)
    l = r2.astype(bf)
    return h, m, l


def _features(x1, x2):
    """Per-batch feature matrices (KF=45 bf16 rows) such that the K-contracted
    matmul A^T B reproduces the fp32 product sum exactly: for each of the 5
    base features f, rows (f,i,j) hold splitA_i[f] / splitB_j[f] so that
    sum_ij Ai*Bj == A[f]*B[f] with bf16-exact cross products."""
    x1 = np.ascontiguousarray(x1, dtype=np.float32)
    x2 = np.ascontiguousarray(x2, dtype=np.float32)
    sq1 = (x1 * x1).sum(-1, dtype=np.float32)       # (B, N)
    sq2 = (x2 * x2).sum(-1, dtype=np.float32)       # (B, M)
    ones1 = np.ones_like(sq1)
    ones2 = np.ones_like(sq2)
    # A[b] rows: [-x, -y, -z, -1, -sq1]  (B, 5, N)
    A = -np.stack([x1[..., 0], x1[..., 1], x1[..., 2], ones1, sq1], axis=1)
    # B[b] rows: [-2x', -2y', -2z', sq2, 1]  (B, 5, M)
    Bf = np.stack(
        [-2.0 * x2[..., 0], -2.0 * x2[..., 1], -2.0 * x2[..., 2], sq2, ones2],
        axis=1,
    ).astype(np.float32)
    A = A.astype(np.float32)

    Ah, Am, Al = _split3(A)           # each (B, 5, N) bf16
    Bh, Bm, Bl = _split3(Bf)
    Asp = np.stack([Ah, Am, Al], axis=2)   # (B, 5, 3, N)
    Bsp = np.stack([Bh, Bm, Bl], axis=2)   # (B, 5, 3, M)
    # Accumulate smallest-magnitude cross terms first (PSUM adds in K order)
    order = [(2, 2), (1, 2), (2, 1), (1, 1), (0, 2), (2, 0), (0, 1), (1, 0),
             (0, 0)]
    A45 = np.concatenate([Asp[:, :, i, :] for (i, j) in order], axis=1)
    B45 = np.concatenate([Bsp[:, :, j, :] for (i, j) in order], axis=1)
    return A45, B45


def _unpack_k(pk):
    """pk: [PT, NT] packed fp32 for one (batch, orientation).
    Returns k [N] int32 (folded column index per row n = t*PT + p)."""
    u = np.ascontiguousarray(pk).view(np.uint32).reshape(PT, NT)
    m = (u & IDXMASK).astype(np.int32)                    # [PT, NT]
    # row n = t*PT + p  ->  [NT, PT] -> flatten
    return np.ascontiguousarray(m.T).reshape(N).astype(np.int32)


def _resolve(xq, xr, sqq, sqr, k):
    """Resolve fold candidates exactly: for each query row xq[b,n], candidates
    m = k[b,n] + SCANW*j (j<FOLD) in xr; pick argmin of exact fp32 distance
    (ties -> smallest m, matching jnp.argmin). Returns (dist, idx)."""
    Bn, Nn = k.shape
    brow = np.arange(Bn)[:, None, None]
    cand = k[:, :, None] + SCANW * np.arange(FOLD)[None, None, :]  # (B,N,FOLD)
    g = xr[brow, cand]                                    # (B,N,FOLD,3)
    d = (
        sqq[:, :, None] + sqr[brow, cand]
        - 2.0 * np.einsum("bnd,bnfd->bnf", xq, g, dtype=np.float32)
    ).astype(np.float32)
    j = np.argmin(d, axis=2)                              # first-min tie-break
    idx = np.take_along_axis(cand, j[:, :, None], axis=2)[:, :, 0]
    dist = np.take_along_axis(d, j[:, :, None], axis=2)[:, :, 0]
    return dist.astype(np.float32), idx.astype(np.int32)


def _run(input1, input2, trace=False):
    from concourse.bass_utils import run_bass_kernel_spmd

    if "nc" not in _CACHE:
        _CACHE["nc"] = _build_program()
    nc = _CACHE["nc"]

    x1 = np.ascontiguousarray(np.asarray(input1), dtype=np.float32)
    x2 = np.ascontiguousarray(np.asarray(input2), dtype=np.float32)
    A, Bf = _features(x1, x2)

    idxf = (
        np.uint32(0x3F800000) + np.arange(SCANW, dtype=np.uint32)
    ).view(np.float32)
    idxf = np.broadcast_to(idxf, (PT, SCANW)).copy()
    maskc = np.full((PT, 1), IDXMASK, np.uint32).view(np.float32)

    in_maps = []
    for c in range(NCORES):
        sl = slice(c * BPC, (c + 1) * BPC)
        # (BPC, KF, N) -> (KF, BPC*N) with [k, b*N + n] layout
        a_np = np.ascontiguousarray(A[sl].transpose(1, 0, 2).reshape(KF, BPC * N))
        b_np = np.ascontiguousarray(Bf[sl].transpose(1, 0, 2).reshape(KF, BPC * M))
        in_maps.append(
            {"feat_a": a_np, "feat_b": b_np, "idxf": idxf, "maskc": maskc}
        )

    res = run_bass_kernel_spmd(nc, in_maps, list(range(NCORES)), trace=trace)

    k1 = np.empty((B, N), np.int32)
    k2 = np.empty((B, M), np.int32)
    for c in range(NCORES):
        r = np.asarray(res.results[c]["packed"], np.float32)
        for bi in range(BPC):
            b = c * BPC + bi
            k1[b] = _unpack_k(r[bi * 2 + 0])
            k2[b] = _unpack_k(r[bi * 2 + 1])

    # exact distances + candidate resolution (fp32, matching reference)
    sq1 = (x1 * x1).sum(-1, dtype=np.float32)            # (B, N)
    sq2 = (x2 * x2).sum(-1, dtype=np.float32)            # (B, M)
    dist1, idx1 = _resolve(x1, x2, sq1, sq2, k1)
    dist2, idx2 = _resolve(x2, x1, sq2, sq1, k2)

    return (dist1, dist2, idx1, idx2), res


def kernel(input1, input2):
    outs, _ = _run(input1, input2, trace=False)
    return outs


def kernel_profiled(input1, input2):
    outs, res = _run(input1, input2, trace=True)
    return outs, res


# revision 39
# speedup vs baseline: 1.0002x; 1.0002x over previous
"""Chamfer distance kernel for Trainium2 (Bass/Tile), 8 NeuronCores.

Problem: B=16 batches of point-cloud pairs (N=M=4096 points, 3-D).
  d[b,n,m] = |x1[b,n]|^2 + |x2[b,m]|^2 - 2*x1[b,n].x2[b,m]
  dist1/idx1 = min/argmin over m, dist2/idx2 = min/argmin over n.

Sharding: data-parallel over batch; each of the 8 cores handles 2 batches.

Device algorithm (per core, per batch, per orientation):
  Features A = -[x; y; z; 1; |p|^2], B = [-2x; -2y; -2z; |p|^2; 1] so a
  K-contracted matmul produces NEGATED squared distances s = -d. Each fp32
  feature is split exactly into 3 bf16 components and all 9 cross terms
  stacked along K (KF=45) so the bf16 matmul reproduces fp32-accurate
  products. PE writes s into [128, 2048] PSUM tiles (8 x 512-col matmuls
  per 128-row tile, 2 PSUM tiles ping-pong).

  The (otherwise idle) Scalar engine drains each PSUM tile into a full-row
  SBUF buffer [128, 4096] as FP16 (10-bit mantissa -> the upconverted fp32
  bits [0..12] are always zero, leaving room for a packed index field).
  The DVE does a position-preserving 4x fold with two stock tensor_tensor
  MAX ops — fp16 qualifies for the 2x_1p mode (2 elem/cycle) — then a
  single custom DVE op (ARGMIN_PACK_ANT) reduces the folded [128, 1024]
  row in ONE pass: key = (s & ~0x3FF) | (col & 0x3FF) packs value and
  column via BITWISE AND/XOR/OR (idx bits streamed as Src1 =
  bitcast(0x3F800000+col), mask as a raw-bit [P,1] scalar); a running
  float-max yields the min bucket and its first folded column k (IEEE
  ordering gives smallest-index tie-break for negative values). True
  column = k + 1024*j, j<4; the host resolves the 4 candidates and the
  output distances exactly from the original inputs (O(B*N)).

  Engine budget per core (measured): Scalar copies 524288 elem/lane at
  1 elem/cycle/1.2 GHz = 502 us and runs gap-free back-to-back (the
  pacer), PE streams bf16 matmul columns at a fixed 1.2 GHz (427 ns per
  512-col matmul, unrampable) = 437 us, DVE fold+scan = 431 us. The
  early-tile prefetch (NEARLY=4 row-tiles of lhs + full b0 rhs) keeps the
  PE fed while the big feature DMAs land. Pipeline lands at ~526 us vs
  1142 us baseline (599 us for the earlier no-fold fp32 scan variant);
  remaining span - scalar busy is ~24 us of fixed NEFF init/teardown.
"""

import sys

import numpy as np

for _p in ("/opt/trn_rl_repo", "/root/.axon_site/_ro/trn_rl_repo"):
    if _p not in sys.path:
        sys.path.append(_p)

B, N, M, D = 16, 4096, 4096, 3
NCORES = 8
BPC = B // NCORES          # batches per core
PT = 128                   # partition tile (output rows per matmul)
FC = 512                   # free-dim chunk (PSUM bank width in fp32)
HALF = 2048                # columns per PSUM tile
NT = N // PT               # 32 row tiles
KF = 45                    # 5 features x 9 bf16-split cross terms (exact fp32)
FOLD = 4                   # fold factor: scan M//FOLD cols, host resolves FOLD candidates
SCANW = M // FOLD          # 1024 columns scanned per row tile
IDXBITS = 10               # low mantissa bits carrying the column index
IDXMASK = (1 << IDXBITS) - 1

_CACHE = {}


def _register_op():
    import concourse.dve_ops as dops
    from concourse.dve_spec import Spec, Src0, Src1, C0, AluOp, Bin, maxx, MaxNeg

    for o in dops.OPS:
        if o.name == "ARGMIN_PACK_ANT":
            return o

    t = Bin(AluOp.BITWISE_AND, Src0, C0)
    u = Bin(AluOp.BITWISE_XOR, Src0, t)
    w = Bin(AluOp.BITWISE_AND, Src1, C0)
    key = Bin(AluOp.BITWISE_OR, u, w)

    def _ref(in0, in1, c0, c1, c2):
        v = np.ascontiguousarray(in0, np.float32).view(np.uint32)
        i = np.ascontiguousarray(in1, np.float32).view(np.uint32)
        m = np.ascontiguousarray(c0, np.float32).view(np.uint32).reshape(-1, 1)
        keyb = ((v & ~m) | (i & m)).view(np.float32)
        P = keyb.shape[0]
        acc = keyb.reshape(P, -1).max(axis=-1, keepdims=True)
        return keyb, acc

    spec = Spec(body=key, accum=maxx, accum_init=MaxNeg, reference=_ref)
    row = max(dops._SUB_OPCODE_FOR_NAME.values()) + 1
    assert row < 0x20
    op = dops.DveOp(
        "ARGMIN_PACK_ANT",
        spec,
        subdim=False,
        uops_sha={"v3": "bac1e8a7ec25ac9f", "v4": "4071d7dc865a2b50"},
    )
    dops.OPS.append(op)
    dops.CUSTOM_DVE_SPECS[op.name] = op.spec
    dops._SUB_OPCODE_FOR_NAME[op.name] = row
    return op


def _build_program():
    import concourse.mybir as mybir
    from concourse import bacc, tile
    from concourse.alu_op_type import AluOpType

    op = _register_op()

    fp32 = mybir.dt.float32
    bf16 = mybir.dt.bfloat16
    fp16 = mybir.dt.float16

    nc = bacc.Bacc(None, target_bir_lowering=False)

    a_dram = nc.dram_tensor("feat_a", [KF, BPC * N], bf16, kind="ExternalInput")
    b_dram = nc.dram_tensor("feat_b", [KF, BPC * M], bf16, kind="ExternalInput")
    idxf_dram = nc.dram_tensor("idxf", [PT, SCANW], fp32, kind="ExternalInput")
    mask_dram = nc.dram_tensor("maskc", [PT, 1], fp32, kind="ExternalInput")
    # packed argmin results: [bi*2+orient, partition, tile]
    pk_dram = nc.dram_tensor(
        "packed", [BPC * 2, PT, NT], fp32, kind="ExternalOutput"
    )

    with tile.TileContext(nc) as tc:
        with (
            tc.tile_pool(name="feat", bufs=1) as featp,
            tc.tile_pool(name="rows", bufs=4) as rowp,
            tc.tile_pool(name="fold", bufs=3) as foldp,
            tc.tile_pool(name="scr", bufs=3) as scrp,
            tc.tile_pool(name="res", bufs=3) as resp,
            tc.tile_pool(name="mm", bufs=2, space="PSUM") as mmp,
        ):
            a_sb = featp.tile([KF, BPC * N], bf16, tag="a")
            b_sb = featp.tile([KF, BPC * M], bf16, tag="b")
            idxf = featp.tile([PT, SCANW], fp32, tag="idxf")
            maskc = featp.tile([PT, 1], fp32, tag="maskc")
            # Small early tiles for the first FOUR row-tiles' matmuls, so PE
            # work starts ~1 us in and stays fed until the big feature DMAs
            # land (b0 is the full rhs for batch 0 / orientation 0).
            NEARLY = 4
            a0_sb = featp.tile([KF, NEARLY * PT], bf16, tag="a0")
            b0_sb = featp.tile([KF, M], bf16, tag="b0")
            # DMA order = need order: tile-0 operands first (PE can start
            # ~1us in), then the big feature tensors (needed from tile 1),
            # then the scan constants (first consumed ~20us in).
            nc.sync.dma_start(a0_sb[:], a_dram[:, 0:NEARLY * PT])
            nc.sync.dma_start(b0_sb[:], b_dram[:, 0:M])
            nc.sync.dma_start(a_sb[:], a_dram[:])
            nc.sync.dma_start(b_sb[:], b_dram[:])
            nc.sync.dma_start(idxf[:], idxf_dram[:])
            nc.sync.dma_start(maskc[:], mask_dram[:])

            # PE warmup: a few tiny matmuls so the tensor engine leaves its
            # cold p-state before the first real matmul group arrives.
            wl = featp.tile([1, PT], bf16, tag="warm_l")
            wr = featp.tile([1, FC], bf16, tag="warm_r")
            nc.gpsimd.memset(wl[:], 0)
            nc.gpsimd.memset(wr[:], 0)
            wm = mmp.tile([PT, HALF], fp32, tag="mm")
            for _ in range(3):
                nc.tensor.matmul(wm[:, 0:FC], wl[:], wr[:], start=True, stop=True)

            for bi in range(BPC):
                for orient in range(2):
                    if orient == 0:
                        lhs_all = a_sb[:, bi * N:(bi + 1) * N]
                        rhs_all = b_sb[:, bi * M:(bi + 1) * M]
                    else:
                        lhs_all = b_sb[:, bi * M:(bi + 1) * M]
                        rhs_all = a_sb[:, bi * N:(bi + 1) * N]

                    res = resp.tile([PT, NT], fp32, tag="res")

                    for t in range(NT):
                        if bi == 0 and orient == 0 and t < NEARLY:
                            lhsT = a0_sb[:, t * PT:(t + 1) * PT]
                            rhs_src = b0_sb
                        else:
                            lhsT = lhs_all[:, t * PT:(t + 1) * PT]
                            rhs_src = None
                        rowbuf = rowp.tile([PT, M], fp16, tag="rowbuf")
                        for h in range(2):
                            mm = mmp.tile([PT, HALF], fp32, tag="mm")
                            rsrc = rhs_src if rhs_src is not None else rhs_all
                            for q in range(4):
                                c = h * 4 + q
                                nc.tensor.matmul(
                                    mm[:, q * FC:(q + 1) * FC],
                                    lhsT,
                                    rsrc[:, c * FC:(c + 1) * FC],
                                    start=True,
                                    stop=True,
                                )
                            # idle Scalar engine drains PSUM -> SBUF fp16
                            # (10-bit mantissa: fp32 bits [0..12] stay zero,
                            # leaving room for the packed index field) and
                            # frees PSUM banks quickly for the PE.
                            nc.scalar.copy(
                                rowbuf[:, h * HALF:(h + 1) * HALF], mm[:]
                            )
                        # position-preserving 4x fold at the TT 2x_1p fp16
                        # rate (2 elem/cycle), then scan only M/4 columns.
                        # Final column = k + SCANW*j for j in 0..3; host
                        # resolves the 4 candidates exactly.
                        f1 = foldp.tile([PT, M // 2], fp16, tag="f1")
                        nc.vector.tensor_tensor(
                            f1[:],
                            rowbuf[:, 0:M // 2],
                            rowbuf[:, M // 2:M],
                            op=AluOpType.max,
                        )
                        f2 = foldp.tile([PT, SCANW], fp16, tag="f2")
                        nc.vector.tensor_tensor(
                            f2[:],
                            f1[:, 0:SCANW],
                            f1[:, SCANW:M // 2],
                            op=AluOpType.max,
                        )
                        scratch = scrp.tile([PT, SCANW], fp32, tag="scratch")
                        nc.vector._custom_dve(
                            op,
                            out=scratch[:],
                            in0=f2[:],
                            in1=idxf[:],
                            s0=maskc[:],
                            accum_out=res[:, t:t + 1],
                        )

                    nc.sync.dma_start(pk_dram[bi * 2 + orient], res[:])

    nc.compile()
    return nc


def _split3(x):
    """Exact 3-way bf16 decomposition of fp32: x == h + m + l."""
    import ml_dtypes

    bf = ml_dtypes.bfloat16
    h = x.astype(bf)
    r1 = (x - h.astype(np.float32)).astype(np.float32)
    m = r1.astype(bf)
    r2 = (r1 - m.astype(np.float32)).astype(np.float32)
    l = r2.astype(bf)
    return h, m, l


def _features(x1, x2):
    """Per-batch feature matrices (KF=45 bf16 rows) such that the K-contracted
    matmul A^T B reproduces the fp32 product sum exactly: for each of the 5
    base features f, rows (f,i,j) hold splitA_i[f] / splitB_j[f] so that
    sum_ij Ai*Bj == A[f]*B[f] with bf16-exact cross products."""
    x1 = np.ascontiguousarray(x1, dtype=np.float32)
    x2 = np.ascontiguousarray(x2, dtype=np.float32)
    sq1 = (x1 * x1).sum(-1, dtype=np.float32)       # (B, N)
    sq2 = (x2 * x2).sum(-1, dtype=np.float32)       # (B, M)
    ones1 = np.ones_like(sq1)
    ones2 = np.ones_like(sq2)
    # A[b] rows: [-x, -y, -z, -1, -sq1]  (B, 5, N)
    A = -np.stack([x1[..., 0], x1[..., 1], x1[..., 2], ones1, sq1], axis=1)
    # B[b] rows: [-2x', -2y', -2z', sq2, 1]  (B, 5, M)
    Bf = np.stack(
        [-2.0 * x2[..., 0], -2.0 * x2[..., 1], -2.0 * x2[..., 2], sq2, ones2],
        axis=1,
    ).astype(np.float32)
    A = A.astype(np.float32)

    Ah, Am, Al = _split3(A)           # each (B, 5, N) bf16
    Bh, Bm, Bl = _split3(Bf)
    Asp = np.stack([Ah, Am, Al], axis=2)   # (B, 5, 3, N)
    Bsp = np.stack([Bh, Bm, Bl], axis=2)   # (B, 5, 3, M)
    # Accumulate smallest-magnitude cross terms first (PSUM adds in K order)
    order = [(2, 2), (1, 2), (2, 1), (1, 1), (0, 2), (2, 0), (0, 1), (1, 0),
             (0, 0)]
    A45 = np.concatenate([Asp[:, :, i, :] for (i, j) in order], axis=1)
    B45 = np.concatenate([Bsp[:, :, j, :] for (i, j) in order], axis=1)
    return A45, B45


def _unpack_k(pk):
    """pk: [PT, NT] packed fp32 for one (batch, orientation).
    Returns k [N] int32 (folded column index per row n = t*PT + p)."""
    u = np.ascontiguousarray(pk).view(np.uint32).reshape(PT, NT)
    m = (u & IDXMASK).astype(np.int32)                    # [PT, NT]
    # row n = t*PT + p  ->  [NT, PT] -> flatten
    return np.ascontiguousarray(m.T).reshape(N).astype(np.int32)


def _resolve(xq, xr, sqq, sqr, k):
    """Resolve fold candidates exactly: for each query row xq[b,n], candidates
    m = k[b,n] + SCANW*j (j<FOLD) in xr; pick argmin of exact fp32 distance
    (ties -> smallest m, matching jnp.argmin). Returns (dist, idx)."""
    Bn, Nn = k.shape
    brow = np.arange(Bn)[:, None, None]
    cand = k[:, :, None] + SCANW * np.arange(FOLD)[None, None, :]  # (B,N,FOLD)
    g = xr[brow, cand]                                    # (B,N,FOLD,3)
    d = (
        sqq[:, :, None] + sqr[brow, cand]
        - 2.0 * np.einsum("bnd,bnfd->bnf", xq, g, dtype=np.float32)
    ).astype(np.float32)
    j = np.argmin(d, axis=2)                              # first-min tie-break
    idx = np.take_along_axis(cand, j[:, :, None], axis=2)[:, :, 0]
    dist = np.take_along_axis(d, j[:, :, None], axis=2)[:, :, 0]
    return dist.astype(np.float32), idx.astype(np.int32)


def _run(input1, input2, trace=False):
    from concourse.bass_utils import run_bass_kernel_spmd

    if "nc" not in _CACHE:
        _CACHE["nc"] = _build_program()
    nc = _CACHE["nc"]

    x1 = np.ascontiguousarray(np.asarray(input1), dtype=np.float32)
    x2 = np.ascontiguousarray(np.asarray(input2), dtype=np.float32)
    A, Bf = _features(x1, x2)

    idxf = (
        np.uint32(0x3F800000) + np.arange(SCANW, dtype=np.uint32)
    ).view(np.float32)
    idxf = np.broadcast_to(idxf, (PT, SCANW)).copy()
    maskc = np.full((PT, 1), IDXMASK, np.uint32).view(np.float32)

    in_maps = []
    for c in range(NCORES):
        sl = slice(c * BPC, (c + 1) * BPC)
        # (BPC, KF, N) -> (KF, BPC*N) with [k, b*N + n] layout
        a_np = np.ascontiguousarray(A[sl].transpose(1, 0, 2).reshape(KF, BPC * N))
        b_np = np.ascontiguousarray(Bf[sl].transpose(1, 0, 2).reshape(KF, BPC * M))
        in_maps.append(
            {"feat_a": a_np, "feat_b": b_np, "idxf": idxf, "maskc": maskc}
        )

    res = run_bass_kernel_spmd(nc, in_maps, list(range(NCORES)), trace=trace)

    k1 = np.empty((B, N), np.int32)
    k2 = np.empty((B, M), np.int32)
    for c in range(NCORES):
        r = np.asarray(res.results[c]["packed"], np.float32)
        for bi in range(BPC):
            b = c * BPC + bi
            k1[b] = _unpack_k(r[bi * 2 + 0])
            k2[b] = _unpack_k(r[bi * 2 + 1])

    # exact distances + candidate resolution (fp32, matching reference)
    sq1 = (x1 * x1).sum(-1, dtype=np.float32)            # (B, N)
    sq2 = (x2 * x2).sum(-1, dtype=np.float32)            # (B, M)
    dist1, idx1 = _resolve(x1, x2, sq1, sq2, k1)
    dist2, idx2 = _resolve(x2, x1, sq2, sq1, k2)

    return (dist1, dist2, idx1, idx2), res


def kernel(input1, input2):
    outs, _ = _run(input1, input2, trace=False)
    return outs


def kernel_profiled(input1, input2):
    outs, res = _run(input1, input2, trace=True)
    return outs, res
